# revision 19
# baseline (speedup 1.0000x reference)
"""Trainium2 Bass kernel for nn_MoEBlock_30502857736769 (moe_routing).

Math (reference):
    out = sum_k v_k * relu(h @ wi^T + (h @ A_k^T) @ B_k^T) @ wo^T

Key algebraic restructuring (exact, since wo is linear):
    wi0'   = wi + B0 @ A0                  (folded on HOST - weight preprocessing)
    p0     = h @ wi0'^T                    (computed ONCE, shared by both experts)
    t      = h @ [A1; A0]^T                (rank-32 LoRA projection, one matmul)
    diff   = t @ [B1, -B0]^T = l1 - l0     (added via one PSUM matmul per f-tile)
    act    = relu(v0*p0) + relu(v1*(p0 + diff))
    out    = act @ wo^T                    (applied ONCE to the weighted sum)

Sharding: pure data-parallel over the 16384 tokens across the 8 NeuronCores
(weights replicated); no collectives.

512-token chunks halve the per-matmul NX dispatch overhead of stage 1 vs the
256-token variant. PSUM (8 banks) cannot hold 512-token stage-2 accumulators
for the full d_model, so stage 2 runs in two d-half passes that share the
same 4 banks sequentially: pass A (d 0:512) interleaves with the stage-1
f-loop as usual; pass B (d 512:1024) runs as a dense per-token-tile drain
against the 32 act tiles kept in SBUF, with evacuation + fp16 stores
streaming behind it. wo's second d-halves are only needed by pass B, which
also halves chunk-0's HBM supply pressure (all 8 cores stream their weights
simultaneously at the head).

All DRAM tensors are pre-arranged on the host into the exact per-partition
SBUF layout, so every DMA is a plain contiguous copy. DMA triggers cost a
fixed ~0.65us on the issuing engine queue; the head packs (A, x0) into ONE
trigger and orders the weight stream by consumption deadline.
"""

import numpy as np

# Problem constants (hardcoded per harness contract - no spec.json reads).
D_MODEL = 1024
D_FF = 4096
N_CORES = 8
B, S = 8, 2048
TOKENS = B * S            # 16384
T = TOKENS // N_CORES     # 2048 tokens per core

P = 128                   # SBUF/PE partition count


def build_program(v0: float, v1: float, t_per_core: int = T, tc: int = 512):
    """Build + compile the SPMD single-core Bass program.

    DRAM parameter layouts (all fp16; all are [128, ...] partition-major so
    DMAs are contiguous per partition):
      hd  [P, KD, 32+tc]    [A-block | x chunk 0] - one head trigger
      xd  [P, NCH, KD, tc]  hidden-states shard, d-major tiles per chunk
                            (chunk 0 slot unused - it ships in hd)
      wid [P, 16, KD, FS]   (wi + B0@A0)^T, f-sixteenth-major
      wod [P, KF, D]        wo^T, f-tile-major (transferred in d-halves)
      bTb [P, F]            [B_i1^T; -B_i0^T; 0...]  (adds l1-l0, t rows 0:32)
    Only bTb's nonzero payload is transferred (full rows for f-tiles 0-7,
    rows 0:32 beyond); rows 32:128 of cols 1024: are synthesized by gpsimd
    memsets whose one-column overlap with the bTbA DMA region orders them
    after that DMA (keeps them off the measured-span start). The B weights
    are zero-padded to K=128 so the diff matmul has a full-row-extent
    LDWEIGHTS (K<128 loads serialize ~2x against in-flight full-row
    matmuls - measured). tq rows 32:127 are zeroed via DMA from bTb's zero
    rows.
      out [Tc, D]   fp16 output shard (host upcasts to fp32)
    """
    import concourse.mybir as mybir
    import concourse.tile as tile
    from concourse import bacc
    from concourse.bass import ts, ds

    dt = mybir.dt
    AF = mybir.ActivationFunctionType

    D, F = D_MODEL, D_FF
    KD = D // P            # 8 contraction tiles over d_model
    KF = F // P            # 32 tiles over d_ff
    FE = F // 8            # 512 f-columns per wi eighth
    FS = FE // 2           # 256 f-columns (2 f-tiles) per wi sixteenth
    DH = D // 2            # 512 d-columns per stage-2 pass
    NCH = t_per_core // tc # token chunks
    TT = tc // P           # 128-token tiles per chunk
    MD = dt.float16

    assert t_per_core % tc == 0 and tc % P == 0

    nc = bacc.Bacc("TRN2", target_bir_lowering=False, debug=False)

    hd = nc.dram_tensor("hd", [P, KD, 32 + tc], MD, kind="ExternalInput")
    xd = nc.dram_tensor("xd", [P, NCH, KD, tc], MD, kind="ExternalInput")
    wid = nc.dram_tensor("wid", [P, 16, KD, FS], MD, kind="ExternalInput")
    wod = nc.dram_tensor("wod", [P, KF, D], MD, kind="ExternalInput")
    bTb = nc.dram_tensor("bTb", [P, F], MD, kind="ExternalInput")
    out = nc.dram_tensor("out", [t_per_core, D], MD, kind="ExternalOutput")
    AOT = mybir.AluOpType

    with tile.TileContext(nc) as tc_ctx:
        with (
            tc_ctx.tile_pool(name="wi", bufs=1) as wi_pool,
            tc_ctx.tile_pool(name="wo", bufs=1) as wo_pool,
            tc_ctx.tile_pool(name="lora_w", bufs=1) as lw_pool,
            tc_ctx.tile_pool(name="x", bufs=2) as x_pool,
            tc_ctx.tile_pool(name="tcat", bufs=2) as tq_pool,
            tc_ctx.tile_pool(name="act", bufs=33) as act_pool,
            tc_ctx.tile_pool(name="a1", bufs=3) as a1_pool,
            tc_ctx.tile_pool(name="osb", bufs=1) as osb_pool,
            tc_ctx.tile_pool(name="ps1", bufs=3, space="PSUM") as ps1_pool,
            tc_ctx.tile_pool(name="pslora", bufs=1, space="PSUM") as pl_pool,
            tc_ctx.tile_pool(name="ps2", bufs=4, space="PSUM") as ps2_pool,
        ):
            # hd split in two so the A-projection chain (which consumes it
            # kd-incrementally) starts ~1us before the full transfer lands.
            hd_t = lw_pool.tile([P, KD, 32 + tc], MD)
            nc.sync.dma_start(hd_t[:, 0:2, :], hd[:, 0:2, :])
            nc.sync.dma_start(hd_t[:, 2:KD, :], hd[:, 2:KD, :])
            x0_t = hd_t[:, :, 32:32 + tc]

            wi_t = wi_pool.tile([P, 16, KD, FS], MD)  # f-16th-major wi^T
            wo_t = wo_pool.tile([P, KF, D], MD)       # f-tile-major wo^T

            def wi_s16(s, n=1):
                nc.sync.dma_start(
                    wi_t[:, ds(s, n), :, :], wid[:, ds(s, n), :, :]
                )

            def wo_half(w):
                nc.sync.dma_start(wo_t[:, w, 0:DH], wod[:, w, 0:DH])

            tq_tiles = {}

            def prep_tq(ch):
                if ch >= NCH or ch in tq_tiles:
                    return
                tq = tq_pool.tile([P, tc], MD, tag="tcat", name="tq")
                nc.sync.dma_start(tq[32:P, :], bTb[32:P, 0:tc])
                tq_tiles[ch] = tq

            # Head trigger order = consumption-deadline order.
            wi_s16(0)
            bTb_t = lw_pool.tile([P, F], MD)
            nc.sync.dma_start(bTb_t[:, 0:8 * P], bTb[:, 0:8 * P])
            # partition-offset ops are limited to 32 partitions each
            for pb in range(32, P, 32):
                nc.gpsimd.memset(bTb_t[pb:pb + 32, 8 * P - 1:F], 0.0)
            prep_tq(0)
            prep_tq(1)
            wi_s16(1)
            wo_half(0)
            wi_s16(2)
            wo_half(1)
            wi_s16(3)
            wo_half(2); wo_half(3)
            wi_s16(4, 2)
            nc.sync.dma_start(bTb_t[0:32, 8 * P:F], bTb[0:32, 8 * P:F])
            wo_half(4); wo_half(5); wo_half(6)
            next_wi, next_wo = 6, 7
            while next_wi < 16:
                wi_s16(next_wi, 2); next_wi += 2
                for _ in range(4):
                    if next_wo < KF:
                        wo_half(next_wo); next_wo += 1
            while next_wo < KF:
                wo_half(next_wo); next_wo += 1
            # wo second d-halves: only pass B (late in each chunk) needs them
            for q in range(4):
                nc.sync.dma_start(
                    wo_t[:, ds(q * 8, 8), DH:D], wod[:, ds(q * 8, 8), DH:D]
                )

            x_tiles = {0: x0_t}

            def load_x(ch):
                if ch >= NCH or ch in x_tiles:
                    return
                x_t = x_pool.tile([P, KD, tc], MD, tag="x", name="x_t")
                nc.sync.dma_start(x_t[:, :, :], xd[:, ch, :, :])
                x_tiles[ch] = x_t

            def chunk_prologue(ch):
                x_t = x_tiles[ch]
                pl = pl_pool.tile([32, tc], dt.float32, tag="pslora", name="pl")
                for kd in range(KD):
                    nc.tensor.matmul(
                        pl[:, :], hd_t[:, kd, 0:32], x_t[:, kd, :],
                        start=(kd == 0), stop=(kd == KD - 1),
                    )
                tq = tq_tiles[ch]
                nc.scalar.copy(tq[0:32, :], pl[:, :])
                return x_t, tq

            for ch in range(NCH):
                x_t, tq = chunk_prologue(ch)

                # ---- pass-A stage-2 accumulators (d 0:512) ----
                ps2a = [
                    ps2_pool.tile([P, DH], dt.float32, tag="ps2", name="ps2")
                    for _ in range(TT)
                ]

                def emit_s2a(act_prev, fi_prev):
                    for tt in range(TT):
                        nc.tensor.matmul(
                            ps2a[tt][:, :],
                            act_prev[:, ts(tt, P)],
                            wo_t[:, fi_prev, 0:DH],
                            start=(fi_prev == 0), stop=(fi_prev == KF - 1),
                        )

                def emit_bdiff(st):
                    p1_, act_, fi_ = st
                    nc.tensor.matmul(
                        p1_[:, :], bTb_t[:, ts(fi_, P)], tq[:, :],
                        start=False, stop=True, skip_group_check=True,
                    )
                    a1_t = a1_pool.tile([P, tc], MD, tag="a1", name="a1_t")
                    nc.vector.tensor_scalar(
                        a1_t[:, :], p1_[:, :], 0.0, float(v1),
                        AOT.max, AOT.mult,
                    )
                    nc.vector.tensor_add(act_[:, :], act_[:, :], a1_t[:, :])

                acts = []         # all f-tiles' acts, consumed again by pass B
                prev = None       # (p1, act, fi) of f-tile i-1
                s2q = []          # acts awaiting pass-A emission
                for fi in range(KF):
                    p1 = ps1_pool.tile([P, tc], dt.float32, tag="ps1")
                    for kd in range(KD):
                        nc.tensor.matmul(
                            p1[:, :],
                            wi_t[:, fi >> 1, kd, ts(fi & 1, P)],
                            x_t[:, kd, :],
                            start=(kd == 0), stop=(kd == KD - 1),
                        )
                    act_t = act_pool.tile([P, tc], MD, tag="act")
                    if v0 >= 0:
                        nc.scalar.activation(
                            act_t[:, :], p1[:, :], AF.Relu,
                            bias=0.0, scale=float(v0),
                        )
                    else:
                        nc.vector.tensor_scalar(
                            act_t[:, :], p1[:, :], 0.0, float(v0),
                            AOT.max, AOT.mult,
                        )
                    acts.append(act_t)
                    if fi == 8:
                        load_x(ch + 1)
                        prep_tq(ch + 1)
                    if len(s2q) >= 2:
                        emit_s2a(*s2q.pop(0))
                    if prev is not None:
                        emit_bdiff(prev)
                        s2q.append((prev[1], prev[2]))
                    prev = (p1, act_t, fi)
                emit_bdiff(prev)
                s2q.append((prev[1], prev[2]))

                # ---- pass-A drain (tt-major) + pass B + streaming stores.
                #      Each tt: finish pass A, evacuate (DVE), store d-half 0;
                #      pass B reuses the freed bank (pool WAR), runs its
                #      32-matmul chain against the SBUF acts, evacuates on
                #      ACT, stores d-half 1. ----
                osb = osb_pool.tile([P, TT, D], MD, tag="osb")
                for tt in range(TT):
                    for act_prev, fi_prev in s2q:
                        nc.tensor.matmul(
                            ps2a[tt][:, :],
                            act_prev[:, ts(tt, P)],
                            wo_t[:, fi_prev, 0:DH],
                            start=(fi_prev == 0), stop=(fi_prev == KF - 1),
                        )
                    nc.vector.tensor_copy(osb[:, tt, 0:DH], ps2a[tt][:, :])
                    nc.gpsimd.dma_start(
                        out[ds(ch * tc + tt * P, P), 0:DH], osb[:, tt, 0:DH]
                    )
                for tt in range(TT):
                    ps2b = ps2_pool.tile(
                        [P, DH], dt.float32, tag="ps2", name="ps2"
                    )
                    for fi in range(KF):
                        nc.tensor.matmul(
                            ps2b[:, :],
                            acts[fi][:, ts(tt, P)],
                            wo_t[:, fi, DH:D],
                            start=(fi == 0), stop=(fi == KF - 1),
                        )
                    # DVE/ACT halves + two store queues: the overhang after
                    # the final pass-B chain shrinks to ~0.2us of copies.
                    nc.vector.tensor_copy(
                        osb[:, tt, DH:DH + 256], ps2b[:, 0:256]
                    )
                    nc.scalar.copy(osb[:, tt, DH + 256:D], ps2b[:, 256:DH])
                    nc.sync.dma_start(
                        out[ds(ch * tc + tt * P, P), DH:DH + 256],
                        osb[:, tt, DH:DH + 256],
                    )
                    nc.gpsimd.dma_start(
                        out[ds(ch * tc + tt * P, P), DH + 256:D],
                        osb[:, tt, DH + 256:D],
                    )

    nc.compile()
    return nc


_PROGRAM_CACHE = {}


def _get_program(v0: float, v1: float):
    key = (float(v0), float(v1))
    if key not in _PROGRAM_CACHE:
        _PROGRAM_CACHE[key] = build_program(v0, v1)
    return _PROGRAM_CACHE[key]


def prep_inputs(hidden_states, wi_w, wo_w, lora_As, lora_Bs,
                top_k_indices, top_k_values, t_per_core: int = T,
                tc: int = 512):
    """Host-side shard + layout prep. Returns (in_maps, v0, v1)."""
    h = np.ascontiguousarray(np.asarray(hidden_states, dtype=np.float32))
    wi = np.asarray(wi_w, dtype=np.float32)
    wo = np.asarray(wo_w, dtype=np.float32)
    As = np.asarray(lora_As, dtype=np.float32)
    Bs = np.asarray(lora_Bs, dtype=np.float32)
    idx = np.asarray(top_k_indices).astype(np.int64)
    vals = np.asarray(top_k_values, dtype=np.float32)

    i0, i1 = int(idx[0]), int(idx[1])
    v0, v1 = float(vals[0]), float(vals[1])

    D, F = D_MODEL, D_FF
    KD, NCH = D // P, t_per_core // tc
    A0, A1 = As[i0], As[i1]                                      # [16, D]
    B0, B1 = Bs[i0], Bs[i1]                                      # [F, 16]
    # Fold expert-0's LoRA into wi (weight preprocessing): p0 = x @ wi0'^T
    wi0 = wi + B0 @ A0                                           # [F, D]
    wiT = np.ascontiguousarray(wi0.T).astype(np.float16)         # [D, F]
    # f-sixteenth-major per-partition layout [P, 16, KD, FS]
    wid = np.ascontiguousarray(
        wiT.reshape(KD, P, 16, F // 16).transpose(1, 2, 0, 3)
    )
    woT = np.ascontiguousarray(wo.T).astype(np.float16)          # [F, D]
    wod = np.ascontiguousarray(woT.reshape(F // P, P, D).transpose(1, 0, 2))
    aT = np.concatenate([A1, A0], axis=0).T.astype(np.float16)   # [D, 32]
    ad = aT.reshape(KD, P, 32).transpose(1, 0, 2)                # [P, KD, 32]
    bTb = np.zeros((P, F), dtype=np.float16)
    bTb[0:16] = B1.T.astype(np.float16)
    bTb[16:32] = (-B0.T).astype(np.float16)

    tokens = h.reshape(TOKENS, D_MODEL)
    n_cores = TOKENS // t_per_core
    in_maps = []
    for c in range(n_cores):
        shard = tokens[c * t_per_core:(c + 1) * t_per_core]
        xT = np.ascontiguousarray(shard.T).astype(np.float16)    # [D, Tc]
        xd = np.ascontiguousarray(
            xT.reshape(KD, P, NCH, tc).transpose(1, 2, 0, 3)
        )                                                        # [P,NCH,KD,tc]
        hd = np.ascontiguousarray(
            np.concatenate([ad, xd[:, 0]], axis=2)
        )                                                        # [P,KD,32+tc]
        in_maps.append({
            "hd": hd, "xd": xd, "wid": wid, "wod": wod, "bTb": bTb,
        })
    return in_maps, v0, v1


# test.py can flip these to profile the run.
TRACE = False
TRACE_CORES = None
LAST_RESULT = None


def kernel(hidden_states, wi_w, wo_w, lora_As, lora_Bs,
           top_k_indices, top_k_values):
    global LAST_RESULT
    from concourse.bass_utils import run_bass_kernel_spmd

    in_maps, v0, v1 = prep_inputs(
        hidden_states, wi_w, wo_w, lora_As, lora_Bs,
        top_k_indices, top_k_values,
    )
    nc = _get_program(v0, v1)
    res = run_bass_kernel_spmd(
        nc, in_maps, list(range(N_CORES)),
        trace=TRACE, trace_cores=TRACE_CORES,
    )
    LAST_RESULT = res
    out = np.concatenate([r["out"] for r in res.results], axis=0)
    return out.reshape(B, S, D_MODEL).astype(np.float32)


# revision 20
# speedup vs baseline: 1.0024x; 1.0024x over previous
"""Trainium2 Bass kernel for nn_MoEBlock_30502857736769 (moe_routing).

Math (reference):
    out = sum_k v_k * relu(h @ wi^T + (h @ A_k^T) @ B_k^T) @ wo^T

Key algebraic restructuring (exact, since wo is linear):
    wi0'   = wi + B0 @ A0                  (folded on HOST - weight preprocessing)
    p0     = h @ wi0'^T                    (computed ONCE, shared by both experts)
    t      = h @ [A1; A0]^T                (rank-32 LoRA projection, one matmul)
    diff   = t @ [B1, -B0]^T = l1 - l0     (added via one PSUM matmul per f-tile)
    act    = relu(v0*p0) + relu(v1*(p0 + diff))
    out    = act @ wo^T                    (applied ONCE to the weighted sum)

Sharding: pure data-parallel over the 16384 tokens across the 8 NeuronCores
(weights replicated); no collectives.

512-token chunks halve the per-matmul NX dispatch overhead of stage 1 vs the
256-token variant. PSUM (8 banks) cannot hold 512-token stage-2 accumulators
for the full d_model, so stage 2 runs in two d-half passes that share the
same 4 banks sequentially: pass A (d 0:512) interleaves with the stage-1
f-loop as usual; pass B (d 512:1024) runs as a dense per-token-tile drain
against the 32 act tiles kept in SBUF, with evacuation + fp16 stores
streaming behind it. wo's second d-halves are only needed by pass B, which
also halves chunk-0's HBM supply pressure (all 8 cores stream their weights
simultaneously at the head).

All DRAM tensors are pre-arranged on the host into the exact per-partition
SBUF layout, so every DMA is a plain contiguous copy. DMA triggers cost a
fixed ~0.65us on the issuing engine queue; the head packs (A, x0) into ONE
trigger and orders the weight stream by consumption deadline.
"""

import numpy as np

# Problem constants (hardcoded per harness contract - no spec.json reads).
D_MODEL = 1024
D_FF = 4096
N_CORES = 8
B, S = 8, 2048
TOKENS = B * S            # 16384
T = TOKENS // N_CORES     # 2048 tokens per core

P = 128                   # SBUF/PE partition count


def build_program(v0: float, v1: float, t_per_core: int = T, tc: int = 512):
    """Build + compile the SPMD single-core Bass program.

    DRAM parameter layouts (all fp16; all are [128, ...] partition-major so
    DMAs are contiguous per partition):
      hd  [P, KD, 32+tc]    [A-block | x chunk 0] - one head trigger
      xd  [P, NCH, KD, tc]  hidden-states shard, d-major tiles per chunk
                            (chunk 0 slot unused - it ships in hd)
      wid [P, 16, KD, FS]   (wi + B0@A0)^T, f-sixteenth-major
      wod [P, KF, D]        wo^T, f-tile-major (transferred in d-halves)
      bTb [P, F]            [B_i1^T; -B_i0^T; 0...]  (adds l1-l0, t rows 0:32)
    Only bTb's nonzero payload is transferred (full rows for f-tiles 0-7,
    rows 0:32 beyond); rows 32:128 of cols 1024: are synthesized by gpsimd
    memsets whose one-column overlap with the bTbA DMA region orders them
    after that DMA (keeps them off the measured-span start). The B weights
    are zero-padded to K=128 so the diff matmul has a full-row-extent
    LDWEIGHTS (K<128 loads serialize ~2x against in-flight full-row
    matmuls - measured). tq rows 32:127 are zeroed via DMA from bTb's zero
    rows.
      out [Tc, D]   fp16 output shard (host upcasts to fp32)
    """
    import concourse.mybir as mybir
    import concourse.tile as tile
    from concourse import bacc
    from concourse.bass import ts, ds

    dt = mybir.dt
    AF = mybir.ActivationFunctionType

    D, F = D_MODEL, D_FF
    KD = D // P            # 8 contraction tiles over d_model
    KF = F // P            # 32 tiles over d_ff
    FE = F // 8            # 512 f-columns per wi eighth
    FS = FE // 2           # 256 f-columns (2 f-tiles) per wi sixteenth
    DH = D // 2            # 512 d-columns per stage-2 pass
    NCH = t_per_core // tc # token chunks
    TT = tc // P           # 128-token tiles per chunk
    MD = dt.float16

    assert t_per_core % tc == 0 and tc % P == 0

    nc = bacc.Bacc("TRN2", target_bir_lowering=False, debug=False)

    hd = nc.dram_tensor("hd", [P, KD, 32 + tc], MD, kind="ExternalInput")
    xd = nc.dram_tensor("xd", [P, NCH, KD, tc], MD, kind="ExternalInput")
    wid = nc.dram_tensor("wid", [P, 16, KD, FS], MD, kind="ExternalInput")
    wod = nc.dram_tensor("wod", [P, KF, D], MD, kind="ExternalInput")
    bTb = nc.dram_tensor("bTb", [P, F], MD, kind="ExternalInput")
    out = nc.dram_tensor("out", [t_per_core, D], MD, kind="ExternalOutput")
    AOT = mybir.AluOpType

    with tile.TileContext(nc) as tc_ctx:
        with (
            tc_ctx.tile_pool(name="wi", bufs=1) as wi_pool,
            tc_ctx.tile_pool(name="wo", bufs=1) as wo_pool,
            tc_ctx.tile_pool(name="lora_w", bufs=1) as lw_pool,
            tc_ctx.tile_pool(name="x", bufs=2) as x_pool,
            tc_ctx.tile_pool(name="tcat", bufs=2) as tq_pool,
            tc_ctx.tile_pool(name="act", bufs=33) as act_pool,
            tc_ctx.tile_pool(name="a1", bufs=3) as a1_pool,
            tc_ctx.tile_pool(name="osb", bufs=1) as osb_pool,
            tc_ctx.tile_pool(name="ps1", bufs=3, space="PSUM") as ps1_pool,
            tc_ctx.tile_pool(name="pslora", bufs=1, space="PSUM") as pl_pool,
            tc_ctx.tile_pool(name="ps2", bufs=4, space="PSUM") as ps2_pool,
        ):
            # hd split in two so the A-projection chain (which consumes it
            # kd-incrementally) starts ~1us before the full transfer lands.
            hd_t = lw_pool.tile([P, KD, 32 + tc], MD)
            nc.sync.dma_start(hd_t[:, 0:2, :], hd[:, 0:2, :])
            nc.sync.dma_start(hd_t[:, 2:KD, :], hd[:, 2:KD, :])
            x0_t = hd_t[:, :, 32:32 + tc]

            wi_t = wi_pool.tile([P, 16, KD, FS], MD)  # f-16th-major wi^T
            wo_t = wo_pool.tile([P, KF, D], MD)       # f-tile-major wo^T

            def wi_s16(s, n=1):
                nc.sync.dma_start(
                    wi_t[:, ds(s, n), :, :], wid[:, ds(s, n), :, :]
                )

            def wo_half(w):
                nc.sync.dma_start(wo_t[:, w, 0:DH], wod[:, w, 0:DH])

            tq_tiles = {}

            def prep_tq(ch):
                if ch >= NCH or ch in tq_tiles:
                    return
                tq = tq_pool.tile([P, tc], MD, tag="tcat", name="tq")
                nc.sync.dma_start(tq[32:P, :], bTb[32:P, 0:tc])
                tq_tiles[ch] = tq

            # Head trigger order = consumption-deadline order.
            wi_s16(0)
            bTb_t = lw_pool.tile([P, F], MD)
            nc.sync.dma_start(bTb_t[:, 0:8 * P], bTb[:, 0:8 * P])
            # partition-offset ops are limited to 32 partitions each
            for pb in range(32, P, 32):
                nc.gpsimd.memset(bTb_t[pb:pb + 32, 8 * P - 1:F], 0.0)
            prep_tq(0)
            prep_tq(1)
            wi_s16(1)
            wo_half(0)
            wi_s16(2)
            wo_half(1)
            wi_s16(3)
            wo_half(2); wo_half(3)
            wi_s16(4, 2)
            nc.sync.dma_start(bTb_t[0:32, 8 * P:F], bTb[0:32, 8 * P:F])
            wo_half(4); wo_half(5); wo_half(6)
            next_wi, next_wo = 6, 7
            while next_wi < 16:
                wi_s16(next_wi, 2); next_wi += 2
                for _ in range(4):
                    if next_wo < KF:
                        wo_half(next_wo); next_wo += 1
            while next_wo < KF:
                wo_half(next_wo); next_wo += 1
            # wo second d-halves: only pass B (late in each chunk) needs them
            for q in range(4):
                nc.sync.dma_start(
                    wo_t[:, ds(q * 8, 8), DH:D], wod[:, ds(q * 8, 8), DH:D]
                )

            x_tiles = {0: x0_t}

            def load_x(ch):
                if ch >= NCH or ch in x_tiles:
                    return
                x_t = x_pool.tile([P, KD, tc], MD, tag="x", name="x_t")
                nc.sync.dma_start(x_t[:, :, :], xd[:, ch, :, :])
                x_tiles[ch] = x_t

            def chunk_prologue(ch):
                x_t = x_tiles[ch]
                pl = pl_pool.tile([32, tc], dt.float32, tag="pslora", name="pl")
                for kd in range(KD):
                    nc.tensor.matmul(
                        pl[:, :], hd_t[:, kd, 0:32], x_t[:, kd, :],
                        start=(kd == 0), stop=(kd == KD - 1),
                    )
                tq = tq_tiles[ch]
                nc.scalar.copy(tq[0:32, :], pl[:, :])
                return x_t, tq

            for ch in range(NCH):
                x_t, tq = chunk_prologue(ch)

                # ---- pass-A stage-2 accumulators (d 0:512) ----
                ps2a = [
                    ps2_pool.tile([P, DH], dt.float32, tag="ps2", name="ps2")
                    for _ in range(TT)
                ]

                def emit_s2a(act_prev, fi_prev):
                    for tt in range(TT):
                        nc.tensor.matmul(
                            ps2a[tt][:, :],
                            act_prev[:, ts(tt, P)],
                            wo_t[:, fi_prev, 0:DH],
                            start=(fi_prev == 0), stop=(fi_prev == KF - 1),
                        )

                def emit_bdiff(st):
                    p1_, act_, fi_ = st
                    nc.tensor.matmul(
                        p1_[:, :], bTb_t[:, ts(fi_, P)], tq[:, :],
                        start=False, stop=True, skip_group_check=True,
                    )
                    a1_t = a1_pool.tile([P, tc], MD, tag="a1", name="a1_t")
                    nc.vector.tensor_scalar(
                        a1_t[:, :], p1_[:, :], 0.0, float(v1),
                        AOT.max, AOT.mult,
                    )
                    nc.vector.tensor_add(act_[:, :], act_[:, :], a1_t[:, :])

                acts = []         # all f-tiles' acts, consumed again by pass B
                prev = None       # (p1, act, fi) of f-tile i-1
                s2q = []          # acts awaiting pass-A emission
                for fi in range(KF):
                    p1 = ps1_pool.tile([P, tc], dt.float32, tag="ps1")
                    for kd in range(KD):
                        nc.tensor.matmul(
                            p1[:, :],
                            wi_t[:, fi >> 1, kd, ts(fi & 1, P)],
                            x_t[:, kd, :],
                            start=(kd == 0), stop=(kd == KD - 1),
                        )
                    act_t = act_pool.tile([P, tc], MD, tag="act")
                    if v0 >= 0:
                        nc.scalar.activation(
                            act_t[:, :], p1[:, :], AF.Relu,
                            bias=0.0, scale=float(v0),
                        )
                    else:
                        nc.vector.tensor_scalar(
                            act_t[:, :], p1[:, :], 0.0, float(v0),
                            AOT.max, AOT.mult,
                        )
                    acts.append(act_t)
                    if fi == 8:
                        load_x(ch + 1)
                        prep_tq(ch + 1)
                    if len(s2q) >= 2:
                        emit_s2a(*s2q.pop(0))
                    if prev is not None:
                        emit_bdiff(prev)
                        s2q.append((prev[1], prev[2]))
                    prev = (p1, act_t, fi)
                emit_bdiff(prev)
                s2q.append((prev[1], prev[2]))

                # ---- pass-A drain (tt-major) + pass B + streaming stores.
                #      Each tt: finish pass A, evacuate (DVE), store d-half 0;
                #      pass B reuses the freed bank (pool WAR), runs its
                #      32-matmul chain against the SBUF acts, evacuates on
                #      ACT, stores d-half 1. ----
                osb = osb_pool.tile([P, TT, D], MD, tag="osb")
                for tt in range(TT):
                    for act_prev, fi_prev in s2q:
                        nc.tensor.matmul(
                            ps2a[tt][:, :],
                            act_prev[:, ts(tt, P)],
                            wo_t[:, fi_prev, 0:DH],
                            start=(fi_prev == 0), stop=(fi_prev == KF - 1),
                        )
                    nc.vector.tensor_copy(osb[:, tt, 0:DH], ps2a[tt][:, :])
                    nc.gpsimd.dma_start(
                        out[ds(ch * tc + tt * P, P), 0:DH], osb[:, tt, 0:DH]
                    )
                for tt in range(TT):
                    ps2b = ps2_pool.tile(
                        [P, DH], dt.float32, tag="ps2", name="ps2"
                    )
                    for fi in range(KF):
                        nc.tensor.matmul(
                            ps2b[:, :],
                            acts[fi][:, ts(tt, P)],
                            wo_t[:, fi, DH:D],
                            start=(fi == 0), stop=(fi == KF - 1),
                        )
                    nc.scalar.copy(osb[:, tt, DH:D], ps2b[:, :])
                    nc.gpsimd.dma_start(
                        out[ds(ch * tc + tt * P, P), DH:D], osb[:, tt, DH:D]
                    )

    nc.compile()
    return nc


_PROGRAM_CACHE = {}


def _get_program(v0: float, v1: float):
    key = (float(v0), float(v1))
    if key not in _PROGRAM_CACHE:
        _PROGRAM_CACHE[key] = build_program(v0, v1)
    return _PROGRAM_CACHE[key]


def prep_inputs(hidden_states, wi_w, wo_w, lora_As, lora_Bs,
                top_k_indices, top_k_values, t_per_core: int = T,
                tc: int = 512):
    """Host-side shard + layout prep. Returns (in_maps, v0, v1)."""
    h = np.ascontiguousarray(np.asarray(hidden_states, dtype=np.float32))
    wi = np.asarray(wi_w, dtype=np.float32)
    wo = np.asarray(wo_w, dtype=np.float32)
    As = np.asarray(lora_As, dtype=np.float32)
    Bs = np.asarray(lora_Bs, dtype=np.float32)
    idx = np.asarray(top_k_indices).astype(np.int64)
    vals = np.asarray(top_k_values, dtype=np.float32)

    i0, i1 = int(idx[0]), int(idx[1])
    v0, v1 = float(vals[0]), float(vals[1])

    D, F = D_MODEL, D_FF
    KD, NCH = D // P, t_per_core // tc
    A0, A1 = As[i0], As[i1]                                      # [16, D]
    B0, B1 = Bs[i0], Bs[i1]                                      # [F, 16]
    # Fold expert-0's LoRA into wi (weight preprocessing): p0 = x @ wi0'^T
    wi0 = wi + B0 @ A0                                           # [F, D]
    wiT = np.ascontiguousarray(wi0.T).astype(np.float16)         # [D, F]
    # f-sixteenth-major per-partition layout [P, 16, KD, FS]
    wid = np.ascontiguousarray(
        wiT.reshape(KD, P, 16, F // 16).transpose(1, 2, 0, 3)
    )
    woT = np.ascontiguousarray(wo.T).astype(np.float16)          # [F, D]
    wod = np.ascontiguousarray(woT.reshape(F // P, P, D).transpose(1, 0, 2))
    aT = np.concatenate([A1, A0], axis=0).T.astype(np.float16)   # [D, 32]
    ad = aT.reshape(KD, P, 32).transpose(1, 0, 2)                # [P, KD, 32]
    bTb = np.zeros((P, F), dtype=np.float16)
    bTb[0:16] = B1.T.astype(np.float16)
    bTb[16:32] = (-B0.T).astype(np.float16)

    tokens = h.reshape(TOKENS, D_MODEL)
    n_cores = TOKENS // t_per_core
    in_maps = []
    for c in range(n_cores):
        shard = tokens[c * t_per_core:(c + 1) * t_per_core]
        xT = np.ascontiguousarray(shard.T).astype(np.float16)    # [D, Tc]
        xd = np.ascontiguousarray(
            xT.reshape(KD, P, NCH, tc).transpose(1, 2, 0, 3)
        )                                                        # [P,NCH,KD,tc]
        hd = np.ascontiguousarray(
            np.concatenate([ad, xd[:, 0]], axis=2)
        )                                                        # [P,KD,32+tc]
        in_maps.append({
            "hd": hd, "xd": xd, "wid": wid, "wod": wod, "bTb": bTb,
        })
    return in_maps, v0, v1


# test.py can flip these to profile the run.
TRACE = False
TRACE_CORES = None
LAST_RESULT = None


def kernel(hidden_states, wi_w, wo_w, lora_As, lora_Bs,
           top_k_indices, top_k_values):
    global LAST_RESULT
    from concourse.bass_utils import run_bass_kernel_spmd

    in_maps, v0, v1 = prep_inputs(
        hidden_states, wi_w, wo_w, lora_As, lora_Bs,
        top_k_indices, top_k_values,
    )
    nc = _get_program(v0, v1)
    res = run_bass_kernel_spmd(
        nc, in_maps, list(range(N_CORES)),
        trace=TRACE, trace_cores=TRACE_CORES,
    )
    LAST_RESULT = res
    out = np.concatenate([r["out"] for r in res.results], axis=0)
    return out.reshape(B, S, D_MODEL).astype(np.float32)
